# revision 32
# baseline (speedup 1.0000x reference)
"""MultiHeadAttention TRN2 kernel: tensor-parallel over heads across 8 NeuronCores.

Problem (hardcoded): BS=2, QLEN=2048, DIM=1024, NHEADS=16, HEAD=64.
  q = split_heads(x @ q_w.T + q_b) / sqrt(64)
  s = q @ k.T + mask ; w = softmax(s) ; ctx = w @ v
  out = merge_heads(ctx) @ o_w.T + o_b

Sharding: core c computes heads {2c, 2c+1} (rows 128c:128c+128 of q/k/v weights,
cols 128c:128c+128 of o_w).  Each core emits a full-shape bf16 partial of the
output projection; the host sums the 8 partials and adds o_b.

Design notes (v2):
- xt is pre-tiled on the host so every [128, 512] projection tile is one
  contiguous 128KB DMA (the strided version ran at ~45 GB/s and made the
  projection phase DMA-bound).
- Softmax denominators ride along inside the PV matmuls: the stationary
  operand is [V_h | ones] ([128, 65]), so psum row 64 accumulates
  sum(exp(scores)) while rows 0..63 accumulate ctx.  This kills the separate
  ones-matmul denominator pass (256 matmuls) entirely.
- 1/s is computed as exp(-ln(s)) on the Scalar engine -- Ln and Exp share one
  activation table so there are no table reloads; DVE reciprocal would cost
  6.5us per call.  The row [1, 512q*2h] reciprocal is broadcast to the 64
  ctx partitions with a 1-contraction matmul against a ones stationary.
- Attention is software-pipelined at qtile granularity (QTW=512 tokens):
  during step i the tensor queue runs scores(i), PV(i-1), and the
  normalize+output-projection of qtile i-2, so the PE never waits on the
  Scalar engine's exp.
- The two heads' ctx tiles both land on psum partitions 0..64 (65-col
  stationaries force output position 0).  h0 evicts in place; h1 is evicted
  to a staging tile and DMA-shifted to partitions 64..127 so the output
  projection can contract over all 128 local dims in one matmul.
"""

import sys

if "/opt/trn_rl_repo" not in sys.path:
    sys.path.insert(0, "/opt/trn_rl_repo")

import math
from contextlib import ExitStack

import ml_dtypes
import numpy as np

import concourse.bass as bass
import concourse.tile as tile
from concourse import bacc, mybir
from concourse.bass_utils import run_bass_kernel_spmd


# ---- problem constants ----
BS, QLEN, DIM, NHEADS = 2, 2048, 1024, 16
HEAD = DIM // NHEADS            # 64
NTOK = BS * QLEN                # 4096
NCORES = 8
HPC = NHEADS // NCORES          # 2 heads per core
LDIM = HPC * HEAD               # 128 local dims per core
NKCH = DIM // 128               # 8 contraction chunks for projections
NTT = NTOK // 512               # 8 token tiles of 512 for projections
NKT = QLEN // 128               # 16 key tiles per batch
QTW = 512                       # query tile width for attention
NQT = QLEN // QTW               # 4 query tiles per batch
NSTEP = BS * NQT                # 8 qtiles total

DT = mybir.dt.bfloat16          # matmul compute dtype
NPDT = ml_dtypes.bfloat16
F32 = mybir.dt.float32
AF = mybir.ActivationFunctionType

_cache = {}


def build_program(dump_debug=False):
    """Build + compile the single-core SPMD Bass program."""
    nc = bacc.Bacc("TRN2", target_bir_lowering=False, debug=False,
                   num_devices=NCORES)
    dbg_ctu = dbg_rc = dbg_ct = None
    if dump_debug:
        dbg_ctu = nc.dram_tensor("dbg_ctu", [NSTEP, 128, 2 * QTW], DT,
                                 kind="ExternalOutput").ap()
        dbg_rc = nc.dram_tensor("dbg_rc", [NSTEP, 128, QTW], F32,
                                kind="ExternalOutput").ap()
        dbg_ct = nc.dram_tensor("dbg_ct", [NSTEP, 128, QTW], DT,
                                kind="ExternalOutput").ap()

    # host-pretiled x^T, partition-major: per token-group g, partition p
    # holds the 8KB row (c, t) -> one descriptor per partition per group.
    xt = nc.dram_tensor("xt", [NTT, 128, NKCH, 512], DT,
                        kind="ExternalInput").ap()
    wq = nc.dram_tensor("wq", [128, NKCH, LDIM], DT,
                        kind="ExternalInput").ap()
    wk = nc.dram_tensor("wk", [128, NKCH, LDIM], DT,
                        kind="ExternalInput").ap()
    wv = nc.dram_tensor("wv", [128, NKCH, LDIM], DT,
                        kind="ExternalInput").ap()
    wo = nc.dram_tensor("wo", [LDIM, DIM], DT, kind="ExternalInput").ap()
    qb = nc.dram_tensor("qb", [LDIM, 1], F32, kind="ExternalInput").ap()
    kb = nc.dram_tensor("kb", [LDIM, 1], F32, kind="ExternalInput").ap()
    vb = nc.dram_tensor("vb", [LDIM, 1], F32, kind="ExternalInput").ap()
    maskd = nc.dram_tensor("maskd", [128, BS * NKT], F32,
                           kind="ExternalInput").ap()
    out = nc.dram_tensor("out", [NTOK, DIM], DT, kind="ExternalOutput").ap()

    with tile.TileContext(nc) as tc, ExitStack() as ctx:
        singles = ctx.enter_context(tc.tile_pool(name="singles", bufs=1))
        vtpool = ctx.enter_context(tc.tile_pool(name="vt", bufs=4))
        evict = ctx.enter_context(tc.tile_pool(name="evict", bufs=6))
        # PSUM: big (4 banks) scores/psqk, pvp (2 banks) PV accum,
        # outp (2 banks) psv / bc / output projection.
        big = ctx.enter_context(tc.tile_pool(name="big", bufs=2, space="PSUM"))
        pvp = ctx.enter_context(tc.tile_pool(name="pvp", bufs=1, space="PSUM"))
        outp = ctx.enter_context(
            tc.tile_pool(name="outp", bufs=2, space="PSUM"))

        # --- resident SBUF tensors ---
        wq_sb = singles.tile([128, NKCH, LDIM], DT, tag="wq")
        wk_sb = singles.tile([128, NKCH, LDIM], DT, tag="wk")
        wv_sb = singles.tile([128, NKCH, LDIM], DT, tag="wv")
        wo_sb = singles.tile([LDIM, DIM], DT, tag="wo")
        qb_sb = singles.tile([LDIM, 1], F32, tag="qb")
        kb_sb = singles.tile([LDIM, 1], F32, tag="kb")
        vb_sb = singles.tile([LDIM, 1], F32, tag="vb")
        mask_sb = singles.tile([128, BS * NKT], F32, tag="mask")
        # selector stationaries: sel[:,0,:] = row-64 extractor (h0 sums),
        # sel[:,1,:] = row-0 extractor (h1 sums); fully standard matmuls
        sel_sb = singles.tile([128, 2, 128], DT, tag="sel")
        xt_sb = singles.tile([128, NTT, NKCH, 512], DT, tag="xt")
        qt_sb = singles.tile([128, NTOK], DT, tag="qt")
        kt_sb = singles.tile([128, NTOK], DT, tag="kt")
        # v2 (full 128-col stationaries; odd widths mis-load on hw):
        #   h0: [V(64) | ones@64 | zeros]  -> ctx rows 0..63, sums row 64
        #   h1: [ones@0 | zeros | V@64..127] -> ctx rows 64..127, sums row 0
        v2_sb = singles.tile([128, BS, NKT, 2, 128], DT, tag="v2")
        st_sb = singles.tile([128, 2, NKT, 2 * QTW], DT, tag="st")
        # unnormalized ctx + sums evicted from psum (rows 0..64 used)
        ctu_sb = singles.tile([128, 2, 2 * QTW], DT, tag="ctu")
        rc32_sb = singles.tile([128, 2, QTW], F32, tag="rc32")
        ct_sb = singles.tile([128, 2, QTW], DT, tag="ct")

        nc.sync.dma_start(wq_sb[:], wq[:])
        nc.sync.dma_start(xt_sb[:, 0, 0:2], xt[0, :, 0:2])
        nc.sync.dma_start(wk_sb[:], wk[:])
        nc.sync.dma_start(wv_sb[:], wv[:])
        nc.sync.dma_start(xt_sb[:, 0, 2:NKCH], xt[0, :, 2:NKCH])
        nc.sync.dma_start(qb_sb[:], qb[:])
        nc.sync.dma_start(kb_sb[:], kb[:])
        nc.sync.dma_start(vb_sb[:], vb[:])
        nc.sync.dma_start(mask_sb[:], maskd[:])
        nc.sync.dma_start(wo_sb[:], wo[:])
        nc.vector.memset(sel_sb[:], 0.0)
        nc.vector.memset(sel_sb[64:65, 0, 0:64], 1.0)
        nc.vector.memset(sel_sb[0:1, 1, 64:128], 1.0)
        nc.vector.memset(v2_sb[:], 0.0)
        nc.vector.memset(v2_sb[:, :, :, 0, HEAD:HEAD + 1], 1.0)
        nc.vector.memset(v2_sb[:, :, :, 1, 0:1], 1.0)
        for g in range(1, NTT):
            nc.sync.dma_start(xt_sb[:, g], xt[g])

        # --- phase 1: QKV projections, 512-token tile at a time ---
        # psqk comes from the pvp pool (single-buffered; evictions are fast),
        # leaving the big pool free for the scores of qtile 0, which are
        # interleaved into the b1 projection groups below.
        def emit_proj_chunk(g, c, psqk, psv):
            xt_t = xt_sb[:, g, c, :]
            st, sp = (c == 0), (c == NKCH - 1)
            nc.tensor.matmul(psqk[:, 0:512], wq_sb[:, c, :], xt_t,
                             start=st, stop=sp)
            nc.tensor.matmul(psqk[:, 512:1024], wk_sb[:, c, :], xt_t,
                             start=st, stop=sp)
            nc.tensor.matmul(psv[:, 0:512], wv_sb[:, c, :], xt_t,
                             start=st, stop=sp)

        def emit_transpose(b, kt_i, vt_t, t):
            # h0 V -> cols 0..63, h1 V -> cols 64..127 (see v2 layout).
            # b0's are on the critical path for PV(0): split each by kpos
            # half across the sync and scalar queues so they finish early.
            if b == 0:
                for h in range(2):
                    nc.sync.dma_start(
                        v2_sb[:, b, kt_i, h, 64 * h:64 * h + 32],
                        vt_t[64 * h:64 * h + 32, 128 * t:128 * (t + 1)],
                        transpose=True)
                    nc.scalar.dma_start(
                        v2_sb[:, b, kt_i, h, 64 * h + 32:64 * h + HEAD],
                        vt_t[64 * h + 32:64 * h + HEAD,
                             128 * t:128 * (t + 1)],
                        transpose=True)
            else:
                nc.sync.dma_start(
                    v2_sb[:, b, kt_i, 0, 0:HEAD],
                    vt_t[0:HEAD, 128 * t:128 * (t + 1)], transpose=True)
                nc.sync.dma_start(
                    v2_sb[:, b, kt_i, 1, HEAD:128],
                    vt_t[HEAD:128, 128 * t:128 * (t + 1)], transpose=True)

        def finish_proj_group(g, psqk, psv):
            b = g // (NTT // 2)
            gs = slice(512 * g, 512 * (g + 1))
            nc.vector.tensor_scalar_add(qt_sb[:, gs], psqk[:, 0:512],
                                        qb_sb[:, 0:1])
            nc.vector.tensor_scalar_add(kt_sb[:, gs], psqk[:, 512:1024],
                                        kb_sb[:, 0:1])
            vt_t = vtpool.tile([128, 512], DT, tag="vtt")
            nc.vector.tensor_scalar_add(vt_t[:], psv[:, 0:512], vb_sb[:, 0:1])
            # V -> [kpos, dim] transposes (2 per kt, sync queue).  The 64
            # transposes execute serially at ~1.2us each and hog the shared
            # DMA rings, so b1's are deferred into pipeline steps 2-3 --
            # they are not consumed until step 5.
            for t in range(4):
                kt_i = (g % (NTT // 2)) * 4 + t
                emit_transpose(b, kt_i, vt_t, t)

        # --- phase 2: attention + output projection, software-pipelined ---
        def emit_scores(i, kt):
            b, qt = i // NQT, i % NQT
            ks = slice(QLEN * b + 128 * kt, QLEN * b + 128 * (kt + 1))
            qsub = slice(QLEN * b + QTW * qt, QLEN * b + QTW * (qt + 1))
            s_ps = big.tile([128, 1024], F32, tag="big")
            for h in range(2):
                hs = slice(HEAD * h, HEAD * (h + 1))
                nc.tensor.matmul(s_ps[:, 512 * h:512 * (h + 1)],
                                 kt_sb[hs, ks], qt_sb[hs, qsub],
                                 start=True, stop=True,
                                 tile_position=(HEAD * h, 0))
            m_ap = mask_sb[:, b * NKT + kt:b * NKT + kt + 1]
            nc.scalar.activation(st_sb[:, i % 2, kt, :], s_ps[:],
                                 AF.Exp, bias=m_ap)

        def emit_pv(i, kt, ps):
            b = i // NQT
            st0, sp0 = (kt == 0), (kt == NKT - 1)
            for h in range(2):
                nc.tensor.matmul(
                    ps[:, 512 * h:512 * (h + 1)],
                    v2_sb[:, b, kt, h, :],
                    st_sb[:, i % 2, kt, 512 * h:512 * (h + 1)],
                    start=st0, stop=sp0, skip_group_check=True)

        def emit_chain_pre(i, ps):
            # evict unnormalized ctx+sums to SBUF (frees psAB for the next
            # qtile's PV within ~1us), then 1/s = exp(-ln(s)) on scalar
            sl = i % 2
            nc.vector.tensor_copy(ctu_sb[:, sl, :], ps[:, 0:1024])
            if dump_debug:
                nc.sync.dma_start(dbg_ctu[i], ctu_sb[:, sl, :])

        def emit_chain_bc(i):
            # broadcast 1/s to 64 partitions per head, normalize, stage h1
            sl = i % 2
            bc = outp.tile([128, 512], F32, tag="outp")
            nc.tensor.matmul(bc[:], sel_sb[:, 0, :],
                             ctu_sb[:, sl, 0:512], start=True, stop=False,
                             skip_group_check=True)
            nc.tensor.matmul(bc[:], sel_sb[:, 1, :],
                             ctu_sb[:, sl, 512:1024], start=False, stop=True,
                             skip_group_check=True)
            nc.vector.reciprocal_approx_fast(rc32_sb[:, sl, :], bc[:])
            nc.vector.tensor_mul(ct_sb[0:64, sl, :], ctu_sb[0:64, sl, 0:512],
                                 rc32_sb[0:64, sl, :])
            nc.vector.tensor_mul(ct_sb[64:128, sl, :],
                                 ctu_sb[64:128, sl, 512:1024],
                                 rc32_sb[64:128, sl, :])
            if dump_debug:
                nc.sync.dma_start(dbg_rc[i], rc32_sb[:, sl, :])
                nc.sync.dma_start(dbg_ct[i], ct_sb[:, sl, :])

        def emit_outproj(i, tt):
            # output projection for tokens [tok0, tok0+128)
            b, qt = i // NQT, i % NQT
            sl = i % 2
            tok0 = QLEN * b + QTW * qt + 128 * tt
            o_sb = evict.tile([128, 1024], DT, tag="osb")
            for j in range(2):
                o_ps = outp.tile([128, 512], F32, tag="outp")
                nc.tensor.matmul(o_ps[:], ct_sb[:, sl, 128 * tt:128 * (tt + 1)],
                                 wo_sb[:, 512 * j:512 * (j + 1)],
                                 start=True, stop=True)
                nc.vector.tensor_copy(o_sb[:, 512 * j:512 * (j + 1)], o_ps[:])
            nc.gpsimd.dma_start(out[tok0:tok0 + 128, :], o_sb[:])

        # b0 projections, then b1 projections interleaved with qtile-0
        # scores.  b0 groups use the (idle, double-buffered) big pool for
        # psqk so group k+1 does not serialize on group k's evictions; v2/sel
        # memsets are emitted after group 0 so the DVE queue reaches group
        # 0's evictions promptly.
        for g in range(NTT // 2):
            psqk = big.tile([128, 1024], F32, tag="big")
            psv = outp.tile([128, 512], F32, tag="outp")
            for c in range(NKCH):
                emit_proj_chunk(g, c, psqk, psv)
            finish_proj_group(g, psqk, psv)
        for g in range(NTT // 2, NTT):
            psqk = pvp.tile([128, 1024], F32, tag="pv")
            psv = outp.tile([128, 512], F32, tag="outp")
            for c in range(NKCH):
                emit_proj_chunk(g, c, psqk, psv)
                if c % 2 == 1:
                    emit_scores(0, (g - NTT // 2) * 4 + (c - 1) // 2)
            finish_proj_group(g, psqk, psv)

        pv_ps = [None, None]   # psAB tile per qtile parity
        for step in range(1, NSTEP + 2):
            i_s = step          # scores/exp qtile
            i_pv = step - 1     # PV qtile
            i_oc = step - 2     # normalize + output projection qtile
            do_s = i_s < NSTEP
            do_pv = 0 <= i_pv < NSTEP
            do_oc = 0 <= i_oc < NSTEP

            if do_oc:
                emit_chain_pre(i_oc, pv_ps[i_oc % 2])
            ps = None
            if do_pv:
                ps = pvp.tile([128, 1024], F32, tag="pv")
                pv_ps[i_pv % 2] = ps

            # even scores/PV interleave (PV lags scores by one kt so the
            # gpsimd psAB eviction has time to free the accumulator)
            for kt in range(NKT + 1):
                if do_s and kt < NKT:
                    emit_scores(i_s, kt)
                if do_pv and kt >= 1:
                    emit_pv(i_pv, kt - 1, ps)
                if kt == 4 and do_oc:
                    emit_chain_bc(i_oc)
                if do_oc and kt in (7, 9, 11, 13):
                    emit_outproj(i_oc, (kt - 7) // 2)


    nc.compile()
    return nc


def shard_inputs(input, mask, q_w, q_b, k_w, k_b, v_w, v_b, o_w, o_b):
    x = np.asarray(input, np.float32)
    # xt[g, p, c, t] = x[512 g + t, 128 c + p]
    xt = np.ascontiguousarray(
        x.reshape(NTT, 512, NKCH, 128).transpose(0, 3, 2, 1)).astype(NPDT)
    m = np.asarray(mask, np.float32).reshape(BS, NKT, 128)
    maskd = np.ascontiguousarray(m.transpose(2, 0, 1).reshape(128, BS * NKT))
    scale = 1.0 / math.sqrt(HEAD)

    def pmaj(w):  # [1024, 128] -> [128, 8, 128] partition-major
        return np.ascontiguousarray(
            w.reshape(NKCH, 128, LDIM).transpose(1, 0, 2)).astype(NPDT)

    in_maps = []
    for c in range(NCORES):
        L = slice(LDIM * c, LDIM * (c + 1))
        in_maps.append({
            "xt": xt,
            "wq": pmaj((q_w[L, :] * scale).T),
            "wk": pmaj(k_w[L, :].T),
            "wv": pmaj(v_w[L, :].T),
            "wo": np.ascontiguousarray(o_w[:, L].T).astype(NPDT),
            "qb": (q_b[L] * scale).astype(np.float32).reshape(LDIM, 1),
            "kb": k_b[L].astype(np.float32).reshape(LDIM, 1),
            "vb": v_b[L].astype(np.float32).reshape(LDIM, 1),
            "maskd": maskd,
        })
    return in_maps


def run(in_maps, **kw):
    if "nc" not in _cache:
        _cache["nc"] = build_program()
    return run_bass_kernel_spmd(_cache["nc"], in_maps,
                                core_ids=list(range(NCORES)), **kw)


def kernel(input, mask, q_w, q_b, k_w, k_b, v_w, v_b, o_w, o_b,
           bs=BS, qlen=QLEN):
    assert int(bs) == BS and int(qlen) == QLEN
    in_maps = shard_inputs(np.asarray(input), np.asarray(mask),
                           np.asarray(q_w), np.asarray(q_b),
                           np.asarray(k_w), np.asarray(k_b),
                           np.asarray(v_w), np.asarray(v_b),
                           np.asarray(o_w), np.asarray(o_b))
    res = run(in_maps)
    acc = np.zeros((NTOK, DIM), np.float32)
    for r in res.results:
        acc += np.asarray(r["out"], dtype=np.float32)
    acc += np.asarray(o_b, np.float32)[None, :]
    return acc


# revision 33
# speedup vs baseline: 1.2216x; 1.2216x over previous
"""MultiHeadAttention TRN2 kernel: tensor-parallel over heads across 8 NeuronCores.

Problem (hardcoded): BS=2, QLEN=2048, DIM=1024, NHEADS=16, HEAD=64.
  q = split_heads(x @ q_w.T + q_b) / sqrt(64)
  s = q @ k.T + mask ; w = softmax(s) ; ctx = w @ v
  out = merge_heads(ctx) @ o_w.T + o_b

Sharding: core c computes heads {2c, 2c+1} (rows 128c:128c+128 of q/k/v weights,
cols 128c:128c+128 of o_w).  Each core emits a full-shape bf16 partial of the
output projection; the host sums the 8 partials and adds o_b.

Design notes (v2):
- xt is pre-tiled on the host so every [128, 512] projection tile is one
  contiguous 128KB DMA (the strided version ran at ~45 GB/s and made the
  projection phase DMA-bound).
- Softmax denominators ride along inside the PV matmuls: the stationary
  operand is [V_h | ones] ([128, 65]), so psum row 64 accumulates
  sum(exp(scores)) while rows 0..63 accumulate ctx.  This kills the separate
  ones-matmul denominator pass (256 matmuls) entirely.
- 1/s is computed as exp(-ln(s)) on the Scalar engine -- Ln and Exp share one
  activation table so there are no table reloads; DVE reciprocal would cost
  6.5us per call.  The row [1, 512q*2h] reciprocal is broadcast to the 64
  ctx partitions with a 1-contraction matmul against a ones stationary.
- Attention is software-pipelined at qtile granularity (QTW=512 tokens):
  during step i the tensor queue runs scores(i), PV(i-1), and the
  normalize+output-projection of qtile i-2, so the PE never waits on the
  Scalar engine's exp.
- The two heads' ctx tiles both land on psum partitions 0..64 (65-col
  stationaries force output position 0).  h0 evicts in place; h1 is evicted
  to a staging tile and DMA-shifted to partitions 64..127 so the output
  projection can contract over all 128 local dims in one matmul.
"""

import sys

if "/opt/trn_rl_repo" not in sys.path:
    sys.path.insert(0, "/opt/trn_rl_repo")

import math
from contextlib import ExitStack

import ml_dtypes
import numpy as np

import concourse.bass as bass
import concourse.tile as tile
from concourse import bacc, mybir
from concourse.bass_utils import run_bass_kernel_spmd


# ---- problem constants ----
BS, QLEN, DIM, NHEADS = 2, 2048, 1024, 16
HEAD = DIM // NHEADS            # 64
NTOK = BS * QLEN                # 4096
NCORES = 8
HPC = NHEADS // NCORES          # 2 heads per core
LDIM = HPC * HEAD               # 128 local dims per core
NKCH = DIM // 128               # 8 contraction chunks for projections
NTT = NTOK // 512               # 8 token tiles of 512 for projections
NKT = QLEN // 128               # 16 key tiles per batch
QTW = 512                       # query tile width for attention
NQT = QLEN // QTW               # 4 query tiles per batch
NSTEP = BS * NQT                # 8 qtiles total

DT = mybir.dt.bfloat16          # matmul compute dtype
NPDT = ml_dtypes.bfloat16
F32 = mybir.dt.float32
AF = mybir.ActivationFunctionType

_cache = {}


def build_program(dump_debug=False):
    """Build + compile the single-core SPMD Bass program."""
    nc = bacc.Bacc("TRN2", target_bir_lowering=False, debug=False,
                   num_devices=NCORES)
    dbg_ctu = dbg_rc = dbg_ct = None
    if dump_debug:
        dbg_ctu = nc.dram_tensor("dbg_ctu", [NSTEP, 128, 2 * QTW], DT,
                                 kind="ExternalOutput").ap()
        dbg_rc = nc.dram_tensor("dbg_rc", [NSTEP, 128, QTW], F32,
                                kind="ExternalOutput").ap()
        dbg_ct = nc.dram_tensor("dbg_ct", [NSTEP, 128, QTW], DT,
                                kind="ExternalOutput").ap()

    # host-pretiled x^T, partition-major: per token-group g, partition p
    # holds the 8KB row (c, t) -> one descriptor per partition per group.
    xt = nc.dram_tensor("xt", [NTT, 128, NKCH, 512], DT,
                        kind="ExternalInput").ap()
    wq = nc.dram_tensor("wq", [128, NKCH, LDIM], DT,
                        kind="ExternalInput").ap()
    wk = nc.dram_tensor("wk", [128, NKCH, LDIM], DT,
                        kind="ExternalInput").ap()
    wv = nc.dram_tensor("wv", [128, NKCH, LDIM], DT,
                        kind="ExternalInput").ap()
    wo = nc.dram_tensor("wo", [LDIM, DIM], DT, kind="ExternalInput").ap()
    qb = nc.dram_tensor("qb", [LDIM, 1], F32, kind="ExternalInput").ap()
    kb = nc.dram_tensor("kb", [LDIM, 1], F32, kind="ExternalInput").ap()
    vb = nc.dram_tensor("vb", [LDIM, 1], F32, kind="ExternalInput").ap()
    maskd = nc.dram_tensor("maskd", [128, BS * NKT], F32,
                           kind="ExternalInput").ap()
    out = nc.dram_tensor("out", [NTOK, DIM], DT, kind="ExternalOutput").ap()

    with tile.TileContext(nc) as tc, ExitStack() as ctx:
        singles = ctx.enter_context(tc.tile_pool(name="singles", bufs=1))
        vtpool = ctx.enter_context(tc.tile_pool(name="vt", bufs=4))
        evict = ctx.enter_context(tc.tile_pool(name="evict", bufs=6))
        # PSUM: big (4 banks) scores/psqk, pvp (2 banks) PV accum,
        # outp (2 banks) psv / bc / output projection.
        big = ctx.enter_context(tc.tile_pool(name="big", bufs=2, space="PSUM"))
        pvp = ctx.enter_context(tc.tile_pool(name="pvp", bufs=1, space="PSUM"))
        outp = ctx.enter_context(
            tc.tile_pool(name="outp", bufs=2, space="PSUM"))

        # --- resident SBUF tensors ---
        wq_sb = singles.tile([128, NKCH, LDIM], DT, tag="wq")
        wk_sb = singles.tile([128, NKCH, LDIM], DT, tag="wk")
        wv_sb = singles.tile([128, NKCH, LDIM], DT, tag="wv")
        wo_sb = singles.tile([LDIM, DIM], DT, tag="wo")
        qb_sb = singles.tile([LDIM, 1], F32, tag="qb")
        kb_sb = singles.tile([LDIM, 1], F32, tag="kb")
        vb_sb = singles.tile([LDIM, 1], F32, tag="vb")
        mask_sb = singles.tile([128, BS * NKT], F32, tag="mask")
        # selector stationaries: sel[:,0,:] = row-64 extractor (h0 sums),
        # sel[:,1,:] = row-0 extractor (h1 sums); fully standard matmuls
        sel_sb = singles.tile([128, 2, 128], DT, tag="sel")
        xt_sb = singles.tile([128, NTT, NKCH, 512], DT, tag="xt")
        qt_sb = singles.tile([128, NTOK], DT, tag="qt")
        kt_sb = singles.tile([128, NTOK], DT, tag="kt")
        # v2 (full 128-col stationaries; odd widths mis-load on hw):
        #   h0: [V(64) | ones@64 | zeros]  -> ctx rows 0..63, sums row 64
        #   h1: [ones@0 | zeros | V@64..127] -> ctx rows 64..127, sums row 0
        v2_sb = singles.tile([128, BS, NKT, 2, 128], DT, tag="v2")
        st_sb = singles.tile([128, 2, NKT, 2 * QTW], DT, tag="st")
        # unnormalized ctx + sums evicted from psum (rows 0..64 used)
        ctu_sb = singles.tile([128, 2, 2 * QTW], DT, tag="ctu")
        rc32_sb = singles.tile([128, 2, QTW], F32, tag="rc32")
        ct_sb = singles.tile([128, 2, QTW], DT, tag="ct")

        nc.sync.dma_start(wq_sb[:], wq[:])
        nc.sync.dma_start(xt_sb[:, 0, 0:2], xt[0, :, 0:2])
        nc.sync.dma_start(wk_sb[:], wk[:])
        nc.sync.dma_start(wv_sb[:], wv[:])
        nc.sync.dma_start(xt_sb[:, 0, 2:NKCH], xt[0, :, 2:NKCH])
        nc.sync.dma_start(qb_sb[:], qb[:])
        nc.sync.dma_start(kb_sb[:], kb[:])
        nc.sync.dma_start(vb_sb[:], vb[:])
        nc.sync.dma_start(mask_sb[:], maskd[:])
        nc.sync.dma_start(wo_sb[:], wo[:])
        nc.vector.memset(sel_sb[:], 0.0)
        nc.vector.memset(sel_sb[64:65, 0, 0:64], 1.0)
        nc.vector.memset(sel_sb[0:1, 1, 64:128], 1.0)
        nc.vector.memset(v2_sb[:], 0.0)
        nc.vector.memset(v2_sb[:, :, :, 0, HEAD:HEAD + 1], 1.0)
        nc.vector.memset(v2_sb[:, :, :, 1, 0:1], 1.0)
        for g in range(1, NTT):
            nc.sync.dma_start(xt_sb[:, g], xt[g])

        # --- phase 1: QKV projections, 512-token tile at a time ---
        # psqk comes from the pvp pool (single-buffered; evictions are fast),
        # leaving the big pool free for the scores of qtile 0, which are
        # interleaved into the b1 projection groups below.
        def emit_proj_chunk(g, c, psqk, psv):
            xt_t = xt_sb[:, g, c, :]
            st, sp = (c == 0), (c == NKCH - 1)
            nc.tensor.matmul(psqk[:, 0:512], wq_sb[:, c, :], xt_t,
                             start=st, stop=sp)
            nc.tensor.matmul(psqk[:, 512:1024], wk_sb[:, c, :], xt_t,
                             start=st, stop=sp)
            nc.tensor.matmul(psv[:, 0:512], wv_sb[:, c, :], xt_t,
                             start=st, stop=sp)

        def emit_transpose(b, kt_i, vt_t, t):
            # h0 V -> cols 0..63, h1 V -> cols 64..127 (see v2 layout)
            nc.sync.dma_start(
                v2_sb[:, b, kt_i, 0, 0:HEAD],
                vt_t[0:HEAD, 128 * t:128 * (t + 1)], transpose=True)
            nc.sync.dma_start(
                v2_sb[:, b, kt_i, 1, HEAD:128],
                vt_t[HEAD:128, 128 * t:128 * (t + 1)], transpose=True)

        def finish_proj_group(g, psqk, psv):
            b = g // (NTT // 2)
            gs = slice(512 * g, 512 * (g + 1))
            nc.vector.tensor_scalar_add(qt_sb[:, gs], psqk[:, 0:512],
                                        qb_sb[:, 0:1])
            nc.vector.tensor_scalar_add(kt_sb[:, gs], psqk[:, 512:1024],
                                        kb_sb[:, 0:1])
            vt_t = vtpool.tile([128, 512], DT, tag="vtt")
            nc.vector.tensor_scalar_add(vt_t[:], psv[:, 0:512], vb_sb[:, 0:1])
            # V -> [kpos, dim] transposes (2 per kt, sync queue).  The 64
            # transposes execute serially at ~1.2us each and hog the shared
            # DMA rings, so b1's are deferred into pipeline steps 2-3 --
            # they are not consumed until step 5.
            for t in range(4):
                kt_i = (g % (NTT // 2)) * 4 + t
                emit_transpose(b, kt_i, vt_t, t)

        # --- phase 2: attention + output projection, software-pipelined ---
        def emit_scores(i, kt):
            b, qt = i // NQT, i % NQT
            ks = slice(QLEN * b + 128 * kt, QLEN * b + 128 * (kt + 1))
            qsub = slice(QLEN * b + QTW * qt, QLEN * b + QTW * (qt + 1))
            s_ps = big.tile([128, 1024], F32, tag="big")
            for h in range(2):
                hs = slice(HEAD * h, HEAD * (h + 1))
                nc.tensor.matmul(s_ps[:, 512 * h:512 * (h + 1)],
                                 kt_sb[hs, ks], qt_sb[hs, qsub],
                                 start=True, stop=True,
                                 tile_position=(HEAD * h, 0))
            m_ap = mask_sb[:, b * NKT + kt:b * NKT + kt + 1]
            nc.scalar.activation(st_sb[:, i % 2, kt, :], s_ps[:],
                                 AF.Exp, bias=m_ap)

        def emit_pv(i, kt, ps):
            b = i // NQT
            st0, sp0 = (kt == 0), (kt == NKT - 1)
            for h in range(2):
                nc.tensor.matmul(
                    ps[:, 512 * h:512 * (h + 1)],
                    v2_sb[:, b, kt, h, :],
                    st_sb[:, i % 2, kt, 512 * h:512 * (h + 1)],
                    start=st0, stop=sp0, skip_group_check=True)

        def emit_chain_pre(i, ps):
            # evict unnormalized ctx+sums to SBUF (frees psAB for the next
            # qtile's PV within ~1us), then 1/s = exp(-ln(s)) on scalar
            sl = i % 2
            nc.vector.tensor_copy(ctu_sb[:, sl, :], ps[:, 0:1024])
            if dump_debug:
                nc.sync.dma_start(dbg_ctu[i], ctu_sb[:, sl, :])

        def emit_chain_bc(i):
            # broadcast 1/s to 64 partitions per head, normalize, stage h1
            sl = i % 2
            bc = outp.tile([128, 512], F32, tag="outp")
            nc.tensor.matmul(bc[:], sel_sb[:, 0, :],
                             ctu_sb[:, sl, 0:512], start=True, stop=False,
                             skip_group_check=True)
            nc.tensor.matmul(bc[:], sel_sb[:, 1, :],
                             ctu_sb[:, sl, 512:1024], start=False, stop=True,
                             skip_group_check=True)
            nc.vector.reciprocal_approx_fast(rc32_sb[:, sl, :], bc[:])
            nc.vector.tensor_mul(ct_sb[0:64, sl, :], ctu_sb[0:64, sl, 0:512],
                                 rc32_sb[0:64, sl, :])
            nc.vector.tensor_mul(ct_sb[64:128, sl, :],
                                 ctu_sb[64:128, sl, 512:1024],
                                 rc32_sb[64:128, sl, :])
            if dump_debug:
                nc.sync.dma_start(dbg_rc[i], rc32_sb[:, sl, :])
                nc.sync.dma_start(dbg_ct[i], ct_sb[:, sl, :])

        def emit_outproj(i, tt):
            # output projection for tokens [tok0, tok0+128)
            b, qt = i // NQT, i % NQT
            sl = i % 2
            tok0 = QLEN * b + QTW * qt + 128 * tt
            o_sb = evict.tile([128, 1024], DT, tag="osb")
            for j in range(2):
                o_ps = outp.tile([128, 512], F32, tag="outp")
                nc.tensor.matmul(o_ps[:], ct_sb[:, sl, 128 * tt:128 * (tt + 1)],
                                 wo_sb[:, 512 * j:512 * (j + 1)],
                                 start=True, stop=True)
                nc.vector.tensor_copy(o_sb[:, 512 * j:512 * (j + 1)], o_ps[:])
            nc.gpsimd.dma_start(out[tok0:tok0 + 128, :], o_sb[:])

        # b0 projections, then b1 projections interleaved with qtile-0
        # scores.  b0 groups use the (idle, double-buffered) big pool for
        # psqk so group k+1 does not serialize on group k's evictions; v2/sel
        # memsets are emitted after group 0 so the DVE queue reaches group
        # 0's evictions promptly.
        for g in range(NTT // 2):
            psqk = big.tile([128, 1024], F32, tag="big")
            psv = outp.tile([128, 512], F32, tag="outp")
            for c in range(NKCH):
                emit_proj_chunk(g, c, psqk, psv)
            finish_proj_group(g, psqk, psv)
        for g in range(NTT // 2, NTT):
            psqk = pvp.tile([128, 1024], F32, tag="pv")
            psv = outp.tile([128, 512], F32, tag="outp")
            for c in range(NKCH):
                emit_proj_chunk(g, c, psqk, psv)
                if c % 2 == 1:
                    emit_scores(0, (g - NTT // 2) * 4 + (c - 1) // 2)
            finish_proj_group(g, psqk, psv)

        pv_ps = [None, None]   # psAB tile per qtile parity
        for step in range(1, NSTEP + 2):
            i_s = step          # scores/exp qtile
            i_pv = step - 1     # PV qtile
            i_oc = step - 2     # normalize + output projection qtile
            do_s = i_s < NSTEP
            do_pv = 0 <= i_pv < NSTEP
            do_oc = 0 <= i_oc < NSTEP

            if do_oc:
                emit_chain_pre(i_oc, pv_ps[i_oc % 2])
            ps = None
            if do_pv:
                ps = pvp.tile([128, 1024], F32, tag="pv")
                pv_ps[i_pv % 2] = ps

            # even scores/PV interleave (PV lags scores by one kt so the
            # gpsimd psAB eviction has time to free the accumulator)
            for kt in range(NKT + 1):
                if do_s and kt < NKT:
                    emit_scores(i_s, kt)
                if do_pv and kt >= 1:
                    emit_pv(i_pv, kt - 1, ps)
                if kt == 4 and do_oc:
                    emit_chain_bc(i_oc)
                if do_oc and kt in (7, 9, 11, 13):
                    emit_outproj(i_oc, (kt - 7) // 2)


    nc.compile()
    return nc


def shard_inputs(input, mask, q_w, q_b, k_w, k_b, v_w, v_b, o_w, o_b):
    x = np.asarray(input, np.float32)
    # xt[g, p, c, t] = x[512 g + t, 128 c + p]
    xt = np.ascontiguousarray(
        x.reshape(NTT, 512, NKCH, 128).transpose(0, 3, 2, 1)).astype(NPDT)
    m = np.asarray(mask, np.float32).reshape(BS, NKT, 128)
    maskd = np.ascontiguousarray(m.transpose(2, 0, 1).reshape(128, BS * NKT))
    scale = 1.0 / math.sqrt(HEAD)

    def pmaj(w):  # [1024, 128] -> [128, 8, 128] partition-major
        return np.ascontiguousarray(
            w.reshape(NKCH, 128, LDIM).transpose(1, 0, 2)).astype(NPDT)

    in_maps = []
    for c in range(NCORES):
        L = slice(LDIM * c, LDIM * (c + 1))
        in_maps.append({
            "xt": xt,
            "wq": pmaj((q_w[L, :] * scale).T),
            "wk": pmaj(k_w[L, :].T),
            "wv": pmaj(v_w[L, :].T),
            "wo": np.ascontiguousarray(o_w[:, L].T).astype(NPDT),
            "qb": (q_b[L] * scale).astype(np.float32).reshape(LDIM, 1),
            "kb": k_b[L].astype(np.float32).reshape(LDIM, 1),
            "vb": v_b[L].astype(np.float32).reshape(LDIM, 1),
            "maskd": maskd,
        })
    return in_maps


def run(in_maps, **kw):
    if "nc" not in _cache:
        _cache["nc"] = build_program()
    return run_bass_kernel_spmd(_cache["nc"], in_maps,
                                core_ids=list(range(NCORES)), **kw)


def kernel(input, mask, q_w, q_b, k_w, k_b, v_w, v_b, o_w, o_b,
           bs=BS, qlen=QLEN):
    assert int(bs) == BS and int(qlen) == QLEN
    in_maps = shard_inputs(np.asarray(input), np.asarray(mask),
                           np.asarray(q_w), np.asarray(q_b),
                           np.asarray(k_w), np.asarray(k_b),
                           np.asarray(v_w), np.asarray(v_b),
                           np.asarray(o_w), np.asarray(o_b))
    res = run(in_maps)
    acc = np.zeros((NTOK, DIM), np.float32)
    for r in res.results:
        acc += np.asarray(r["out"], dtype=np.float32)
    acc += np.asarray(o_b, np.float32)[None, :]
    return acc


# revision 34
# speedup vs baseline: 1.2352x; 1.0111x over previous
"""MultiHeadAttention TRN2 kernel: tensor-parallel over heads across 8 NeuronCores.

Problem (hardcoded): BS=2, QLEN=2048, DIM=1024, NHEADS=16, HEAD=64.
  q = split_heads(x @ q_w.T + q_b) / sqrt(64)
  s = q @ k.T + mask ; w = softmax(s) ; ctx = w @ v
  out = merge_heads(ctx) @ o_w.T + o_b

Sharding: core c computes heads {2c, 2c+1} (rows 128c:128c+128 of q/k/v weights,
cols 128c:128c+128 of o_w).  Each core emits a full-shape bf16 partial of the
output projection; the host sums the 8 partials and adds o_b.

Design notes (~224us HW, vs 477us baseline):
- x^T is host-pretiled partition-major so each 512-token projection group is
  ONE DMA of 8KB-contiguous rows per partition (strided loads ran ~45 GB/s
  and made the projection phase DMA-bound); the whole x^T stays in SBUF.
- Softmax denominators ride along inside the PV matmuls: the stationary is a
  full [128, 128] operand (odd widths like [128, 65] mis-load on hardware!)
  holding [V_h0(64) | ones | zeros] for h0 and [ones | zeros | V_h1] for h1,
  so ctx lands on psum rows 0..63 / 64..127 respectively with the exp-sums
  on rows 64 / 0, and no separate denominator matmul pass or cross-partition
  shift is ever needed.
- 1/s: the two sums rows are broadcast to their head's 64 partitions by two
  accumulating matmuls against constant row-selector stationaries (standard
  full-shape matmuls -- PE 32-row tiles also misbehave on hardware), then one
  reciprocal_approx_fast (fp32, ~18 bits) on the DVE.  The scalar engine runs
  ONLY the softmax exp (with the mask via the activation bias port), keeping
  one activation table loaded for the whole kernel.
- Attention is software-pipelined at qtile granularity (QTW=512 tokens):
  during step i the tensor queue interleaves scores(i), PV(i-1) (lagging one
  kt), and the normalize + output projection of qtile i-2, so the PE never
  waits on the scalar engine's exp.  Qtile 0's scores are interleaved into
  the b1 projection groups.  PSUM: scores 2x[128,1024] + PV accumulator
  [128,1024] + psv/bc/outproj 2x[128,512] = exactly 8 banks.
- V is transposed to [kpos, dim] via DMA-transpose on the sync queue; these
  run serially at ~1.2us each and hog the shared DMA rings, so everything
  latency-sensitive (ct evictions, output stores) is issued on other queues
  (gpsimd) and buffered deep enough (evict bufs=6) to ride out the window.
- Output partials are stored in bf16 (halves the 16MB/core store traffic);
  the host accumulates the 8 partials in fp32 and adds o_b.
"""

import sys

if "/opt/trn_rl_repo" not in sys.path:
    sys.path.insert(0, "/opt/trn_rl_repo")

import math
from contextlib import ExitStack

import ml_dtypes
import numpy as np

import concourse.bass as bass
import concourse.tile as tile
from concourse import bacc, mybir
from concourse.bass_utils import run_bass_kernel_spmd


# ---- problem constants ----
BS, QLEN, DIM, NHEADS = 2, 2048, 1024, 16
HEAD = DIM // NHEADS            # 64
NTOK = BS * QLEN                # 4096
NCORES = 8
HPC = NHEADS // NCORES          # 2 heads per core
LDIM = HPC * HEAD               # 128 local dims per core
NKCH = DIM // 128               # 8 contraction chunks for projections
NTT = NTOK // 512               # 8 token tiles of 512 for projections
NKT = QLEN // 128               # 16 key tiles per batch
QTW = 512                       # query tile width for attention
NQT = QLEN // QTW               # 4 query tiles per batch
NSTEP = BS * NQT                # 8 qtiles total

DT = mybir.dt.bfloat16          # matmul compute dtype
NPDT = ml_dtypes.bfloat16
F32 = mybir.dt.float32
AF = mybir.ActivationFunctionType

_cache = {}


def build_program(dump_debug=False):
    """Build + compile the single-core SPMD Bass program."""
    nc = bacc.Bacc("TRN2", target_bir_lowering=False, debug=False,
                   num_devices=NCORES)
    dbg_ctu = dbg_rc = dbg_ct = None
    if dump_debug:
        dbg_ctu = nc.dram_tensor("dbg_ctu", [NSTEP, 128, 2 * QTW], DT,
                                 kind="ExternalOutput").ap()
        dbg_rc = nc.dram_tensor("dbg_rc", [NSTEP, 128, QTW], F32,
                                kind="ExternalOutput").ap()
        dbg_ct = nc.dram_tensor("dbg_ct", [NSTEP, 128, QTW], DT,
                                kind="ExternalOutput").ap()

    # host-pretiled x^T, partition-major: per token-group g, partition p
    # holds the 8KB row (c, t) -> one descriptor per partition per group.
    xt = nc.dram_tensor("xt", [NTT, 128, NKCH, 512], DT,
                        kind="ExternalInput").ap()
    wq = nc.dram_tensor("wq", [128, NKCH, LDIM], DT,
                        kind="ExternalInput").ap()
    wk = nc.dram_tensor("wk", [128, NKCH, LDIM], DT,
                        kind="ExternalInput").ap()
    wv = nc.dram_tensor("wv", [128, NKCH, LDIM], DT,
                        kind="ExternalInput").ap()
    wo = nc.dram_tensor("wo", [LDIM, DIM], DT, kind="ExternalInput").ap()
    qb = nc.dram_tensor("qb", [LDIM, 1], F32, kind="ExternalInput").ap()
    kb = nc.dram_tensor("kb", [LDIM, 1], F32, kind="ExternalInput").ap()
    vb = nc.dram_tensor("vb", [LDIM, 1], F32, kind="ExternalInput").ap()
    maskd = nc.dram_tensor("maskd", [128, BS * NKT], F32,
                           kind="ExternalInput").ap()
    out = nc.dram_tensor("out", [NTOK, DIM], DT, kind="ExternalOutput").ap()

    with tile.TileContext(nc) as tc, ExitStack() as ctx:
        singles = ctx.enter_context(tc.tile_pool(name="singles", bufs=1))
        vtpool = ctx.enter_context(tc.tile_pool(name="vt", bufs=4))
        evict = ctx.enter_context(tc.tile_pool(name="evict", bufs=6))
        # PSUM: big (4 banks) scores/psqk, pvp (2 banks) PV accum,
        # outp (2 banks) psv / bc / output projection.
        big = ctx.enter_context(tc.tile_pool(name="big", bufs=2, space="PSUM"))
        pvp = ctx.enter_context(tc.tile_pool(name="pvp", bufs=1, space="PSUM"))
        outp = ctx.enter_context(
            tc.tile_pool(name="outp", bufs=2, space="PSUM"))

        # --- resident SBUF tensors ---
        wq_sb = singles.tile([128, NKCH, LDIM], DT, tag="wq")
        wk_sb = singles.tile([128, NKCH, LDIM], DT, tag="wk")
        wv_sb = singles.tile([128, NKCH, LDIM], DT, tag="wv")
        wo_sb = singles.tile([LDIM, DIM], DT, tag="wo")
        qb_sb = singles.tile([LDIM, 1], F32, tag="qb")
        kb_sb = singles.tile([LDIM, 1], F32, tag="kb")
        vb_sb = singles.tile([LDIM, 1], F32, tag="vb")
        mask_sb = singles.tile([128, BS * NKT], F32, tag="mask")
        # selector stationaries: sel[:,0,:] = row-64 extractor (h0 sums),
        # sel[:,1,:] = row-0 extractor (h1 sums); fully standard matmuls
        sel_sb = singles.tile([128, 2, 128], DT, tag="sel")
        xt_sb = singles.tile([128, NTT, NKCH, 512], DT, tag="xt")
        qt_sb = singles.tile([128, NTOK], DT, tag="qt")
        kt_sb = singles.tile([128, NTOK], DT, tag="kt")
        # v2 (full 128-col stationaries; odd widths mis-load on hw):
        #   h0: [V(64) | ones@64 | zeros]  -> ctx rows 0..63, sums row 64
        #   h1: [ones@0 | zeros | V@64..127] -> ctx rows 64..127, sums row 0
        v2_sb = singles.tile([128, BS, NKT, 2, 128], DT, tag="v2")
        st_sb = singles.tile([128, 2, NKT, 2 * QTW], DT, tag="st")
        # unnormalized ctx + sums evicted from psum (rows 0..64 used)
        ctu_sb = singles.tile([128, 2, 2 * QTW], DT, tag="ctu")
        rc32_sb = singles.tile([128, 2, QTW], F32, tag="rc32")
        ct_sb = singles.tile([128, 2, QTW], DT, tag="ct")

        nc.sync.dma_start(wq_sb[:], wq[:])
        nc.sync.dma_start(xt_sb[:, 0, 0:2], xt[0, :, 0:2])
        nc.sync.dma_start(wk_sb[:], wk[:])
        nc.sync.dma_start(wv_sb[:], wv[:])
        nc.sync.dma_start(xt_sb[:, 0, 2:NKCH], xt[0, :, 2:NKCH])
        nc.sync.dma_start(qb_sb[:], qb[:])
        nc.sync.dma_start(kb_sb[:], kb[:])
        nc.sync.dma_start(vb_sb[:], vb[:])
        nc.sync.dma_start(mask_sb[:], maskd[:])
        nc.sync.dma_start(wo_sb[:], wo[:])
        nc.vector.memset(sel_sb[:], 0.0)
        nc.vector.memset(sel_sb[64:65, 0, 0:64], 1.0)
        nc.vector.memset(sel_sb[0:1, 1, 64:128], 1.0)
        nc.vector.memset(v2_sb[:], 0.0)
        nc.vector.memset(v2_sb[:, :, :, 0, HEAD:HEAD + 1], 1.0)
        nc.vector.memset(v2_sb[:, :, :, 1, 0:1], 1.0)
        for g in range(1, NTT):
            nc.sync.dma_start(xt_sb[:, g], xt[g])

        # --- phase 1: QKV projections, 512-token tile at a time ---
        # psqk comes from the pvp pool (single-buffered; evictions are fast),
        # leaving the big pool free for the scores of qtile 0, which are
        # interleaved into the b1 projection groups below.
        def emit_proj_chunk(g, c, psqk, psv):
            xt_t = xt_sb[:, g, c, :]
            st, sp = (c == 0), (c == NKCH - 1)
            nc.tensor.matmul(psqk[:, 0:512], wq_sb[:, c, :], xt_t,
                             start=st, stop=sp)
            nc.tensor.matmul(psqk[:, 512:1024], wk_sb[:, c, :], xt_t,
                             start=st, stop=sp)
            nc.tensor.matmul(psv[:, 0:512], wv_sb[:, c, :], xt_t,
                             start=st, stop=sp)

        def emit_transpose(b, kt_i, vt_t, t):
            # h0 V -> cols 0..63, h1 V -> cols 64..127 (see v2 layout)
            nc.sync.dma_start(
                v2_sb[:, b, kt_i, 0, 0:HEAD],
                vt_t[0:HEAD, 128 * t:128 * (t + 1)], transpose=True)
            nc.sync.dma_start(
                v2_sb[:, b, kt_i, 1, HEAD:128],
                vt_t[HEAD:128, 128 * t:128 * (t + 1)], transpose=True)

        def finish_proj_group(g, psqk, psv):
            b = g // (NTT // 2)
            gs = slice(512 * g, 512 * (g + 1))
            nc.vector.tensor_scalar_add(qt_sb[:, gs], psqk[:, 0:512],
                                        qb_sb[:, 0:1])
            nc.vector.tensor_scalar_add(kt_sb[:, gs], psqk[:, 512:1024],
                                        kb_sb[:, 0:1])
            vt_t = vtpool.tile([128, 512], DT, tag="vtt")
            nc.vector.tensor_scalar_add(vt_t[:], psv[:, 0:512], vb_sb[:, 0:1])
            # V -> [kpos, dim] transposes (2 per kt, sync queue).  The 64
            # transposes execute serially at ~1.2us each and hog the shared
            # DMA rings, so b1's are deferred into pipeline steps 2-3 --
            # they are not consumed until step 5.
            for t in range(4):
                kt_i = (g % (NTT // 2)) * 4 + t
                emit_transpose(b, kt_i, vt_t, t)

        # --- phase 2: attention + output projection, software-pipelined ---
        def emit_scores(i, kt):
            b, qt = i // NQT, i % NQT
            ks = slice(QLEN * b + 128 * kt, QLEN * b + 128 * (kt + 1))
            qsub = slice(QLEN * b + QTW * qt, QLEN * b + QTW * (qt + 1))
            s_ps = big.tile([128, 1024], F32, tag="big")
            for h in range(2):
                hs = slice(HEAD * h, HEAD * (h + 1))
                nc.tensor.matmul(s_ps[:, 512 * h:512 * (h + 1)],
                                 kt_sb[hs, ks], qt_sb[hs, qsub],
                                 start=True, stop=True,
                                 tile_position=(HEAD * h, 0))
            m_ap = mask_sb[:, b * NKT + kt:b * NKT + kt + 1]
            nc.scalar.activation(st_sb[:, i % 2, kt, :], s_ps[:],
                                 AF.Exp, bias=m_ap)

        def emit_pv(i, kt, ps):
            b = i // NQT
            st0, sp0 = (kt == 0), (kt == NKT - 1)
            for h in range(2):
                nc.tensor.matmul(
                    ps[:, 512 * h:512 * (h + 1)],
                    v2_sb[:, b, kt, h, :],
                    st_sb[:, i % 2, kt, 512 * h:512 * (h + 1)],
                    start=st0, stop=sp0, skip_group_check=True)

        def emit_chain_pre(i, ps):
            # evict unnormalized ctx+sums to SBUF (frees psAB for the next
            # qtile's PV within ~1us), then 1/s = exp(-ln(s)) on scalar
            sl = i % 2
            nc.vector.tensor_copy(ctu_sb[:, sl, :], ps[:, 0:1024])
            if dump_debug:
                nc.sync.dma_start(dbg_ctu[i], ctu_sb[:, sl, :])

        def emit_chain_bc(i):
            # broadcast 1/s to 64 partitions per head, normalize, stage h1
            sl = i % 2
            bc = outp.tile([128, 512], F32, tag="outp")
            nc.tensor.matmul(bc[:], sel_sb[:, 0, :],
                             ctu_sb[:, sl, 0:512], start=True, stop=False,
                             skip_group_check=True)
            nc.tensor.matmul(bc[:], sel_sb[:, 1, :],
                             ctu_sb[:, sl, 512:1024], start=False, stop=True,
                             skip_group_check=True)
            nc.vector.reciprocal_approx_fast(rc32_sb[:, sl, :], bc[:])
            nc.vector.tensor_mul(ct_sb[0:64, sl, :], ctu_sb[0:64, sl, 0:512],
                                 rc32_sb[0:64, sl, :])
            nc.vector.tensor_mul(ct_sb[64:128, sl, :],
                                 ctu_sb[64:128, sl, 512:1024],
                                 rc32_sb[64:128, sl, :])
            if dump_debug:
                nc.sync.dma_start(dbg_rc[i], rc32_sb[:, sl, :])
                nc.sync.dma_start(dbg_ct[i], ct_sb[:, sl, :])

        def emit_outproj(i, tt):
            # output projection for tokens [tok0, tok0+128)
            b, qt = i // NQT, i % NQT
            sl = i % 2
            tok0 = QLEN * b + QTW * qt + 128 * tt
            o_sb = evict.tile([128, 1024], DT, tag="osb")
            for j in range(2):
                o_ps = outp.tile([128, 512], F32, tag="outp")
                nc.tensor.matmul(o_ps[:], ct_sb[:, sl, 128 * tt:128 * (tt + 1)],
                                 wo_sb[:, 512 * j:512 * (j + 1)],
                                 start=True, stop=True)
                nc.vector.tensor_copy(o_sb[:, 512 * j:512 * (j + 1)], o_ps[:])
            nc.gpsimd.dma_start(out[tok0:tok0 + 128, :], o_sb[:])

        # b0 projections, then b1 projections interleaved with qtile-0
        # scores.  b0 groups use the (idle, double-buffered) big pool for
        # psqk so group k+1 does not serialize on group k's evictions; v2/sel
        # memsets are emitted after group 0 so the DVE queue reaches group
        # 0's evictions promptly.
        for g in range(NTT // 2):
            psqk = big.tile([128, 1024], F32, tag="big")
            psv = outp.tile([128, 512], F32, tag="outp")
            for c in range(NKCH):
                emit_proj_chunk(g, c, psqk, psv)
            finish_proj_group(g, psqk, psv)
        for g in range(NTT // 2, NTT):
            psqk = pvp.tile([128, 1024], F32, tag="pv")
            psv = outp.tile([128, 512], F32, tag="outp")
            for c in range(NKCH):
                emit_proj_chunk(g, c, psqk, psv)
                if c % 2 == 1:
                    emit_scores(0, (g - NTT // 2) * 4 + (c - 1) // 2)
            finish_proj_group(g, psqk, psv)

        pv_ps = [None, None]   # psAB tile per qtile parity
        for step in range(1, NSTEP + 2):
            i_s = step          # scores/exp qtile
            i_pv = step - 1     # PV qtile
            i_oc = step - 2     # normalize + output projection qtile
            do_s = i_s < NSTEP
            do_pv = 0 <= i_pv < NSTEP
            do_oc = 0 <= i_oc < NSTEP

            if do_oc:
                emit_chain_pre(i_oc, pv_ps[i_oc % 2])
            ps = None
            if do_pv:
                ps = pvp.tile([128, 1024], F32, tag="pv")
                pv_ps[i_pv % 2] = ps

            # even scores/PV interleave (PV lags scores by one kt so the
            # gpsimd psAB eviction has time to free the accumulator)
            for kt in range(NKT + 1):
                if do_s and kt < NKT:
                    emit_scores(i_s, kt)
                if do_pv and kt >= 1:
                    emit_pv(i_pv, kt - 1, ps)
                if kt == 4 and do_oc:
                    emit_chain_bc(i_oc)
                if do_oc and kt in (7, 9, 11, 13):
                    emit_outproj(i_oc, (kt - 7) // 2)


    nc.compile()
    return nc


def shard_inputs(input, mask, q_w, q_b, k_w, k_b, v_w, v_b, o_w, o_b):
    x = np.asarray(input, np.float32)
    # xt[g, p, c, t] = x[512 g + t, 128 c + p]
    xt = np.ascontiguousarray(
        x.reshape(NTT, 512, NKCH, 128).transpose(0, 3, 2, 1)).astype(NPDT)
    m = np.asarray(mask, np.float32).reshape(BS, NKT, 128)
    maskd = np.ascontiguousarray(m.transpose(2, 0, 1).reshape(128, BS * NKT))
    scale = 1.0 / math.sqrt(HEAD)

    def pmaj(w):  # [1024, 128] -> [128, 8, 128] partition-major
        return np.ascontiguousarray(
            w.reshape(NKCH, 128, LDIM).transpose(1, 0, 2)).astype(NPDT)

    in_maps = []
    for c in range(NCORES):
        L = slice(LDIM * c, LDIM * (c + 1))
        in_maps.append({
            "xt": xt,
            "wq": pmaj((q_w[L, :] * scale).T),
            "wk": pmaj(k_w[L, :].T),
            "wv": pmaj(v_w[L, :].T),
            "wo": np.ascontiguousarray(o_w[:, L].T).astype(NPDT),
            "qb": (q_b[L] * scale).astype(np.float32).reshape(LDIM, 1),
            "kb": k_b[L].astype(np.float32).reshape(LDIM, 1),
            "vb": v_b[L].astype(np.float32).reshape(LDIM, 1),
            "maskd": maskd,
        })
    return in_maps


def run(in_maps, **kw):
    if "nc" not in _cache:
        _cache["nc"] = build_program()
    return run_bass_kernel_spmd(_cache["nc"], in_maps,
                                core_ids=list(range(NCORES)), **kw)


def kernel(input, mask, q_w, q_b, k_w, k_b, v_w, v_b, o_w, o_b,
           bs=BS, qlen=QLEN):
    assert int(bs) == BS and int(qlen) == QLEN
    in_maps = shard_inputs(np.asarray(input), np.asarray(mask),
                           np.asarray(q_w), np.asarray(q_b),
                           np.asarray(k_w), np.asarray(k_b),
                           np.asarray(v_w), np.asarray(v_b),
                           np.asarray(o_w), np.asarray(o_b))
    res = run(in_maps)
    acc = np.zeros((NTOK, DIM), np.float32)
    for r in res.results:
        acc += np.asarray(r["out"], dtype=np.float32)
    acc += np.asarray(o_b, np.float32)[None, :]
    return acc
